# revision 1
# baseline (speedup 1.0000x reference)
"""Trainium2 Bass kernel for nn_DiffuserAttention (GNN edge-softmax message passing).

Sharding: nodes are renumbered into "slots" by a global bin-packing of dst
nodes into edge-tiles (<=128 edges, <=8 dst nodes per tile); each of the 8
cores owns a contiguous slot range (tiles dealt contiguously). Edge softmax
numerators are folded into per-(tile, head) stationary matmul weights S
(pexp values at host-known one-hot positions); segment sums become tiny PE
matmuls into per-tile PSUM slot ranges. h rows live in HBM as bf16 and are
edge-gathered with dma_gather; each step's shard update is AllGathered.
"""
import contextlib
import math
import numpy as np

B, S, D = 2, 4096, 768
H, HD = 12, 64
N = B * S
ALPHA = 0.1
STEPS = 5
EPS = 1e-12
NCORES = 8

TILE_E = 128      # edges per tile
TILE_S = 8        # dst slots per tile
GROUP_T = 16      # tiles per PSUM group (=> 128 slots per group)
SCHUNK_T = 8      # tiles per score-phase gather chunk (SBUF pressure)


# ---------------------------------------------------------------------------
# Host-side graph preprocessing
# ---------------------------------------------------------------------------

def build_structures(edge_src, edge_dst, n_nodes=N):
    edge_src = np.asarray(edge_src, np.int64)
    edge_dst = np.asarray(edge_dst, np.int64)
    order = np.argsort(edge_dst, kind="stable")
    ssrc = edge_src[order]
    counts = np.bincount(edge_dst, minlength=n_nodes)
    offs = np.concatenate([[0], np.cumsum(counts)])

    tiles, cur, cur_e = [], [], 0
    for node in range(n_nodes):
        deg = int(counts[node])
        if deg > TILE_E:
            raise ValueError("node degree exceeds TILE_E")
        if len(cur) >= TILE_S or cur_e + deg > TILE_E:
            tiles.append(cur)
            cur, cur_e = [], 0
        cur.append(node)
        cur_e += deg
    if cur:
        tiles.append(cur)

    T_g = len(tiles)
    T_core = -(-T_g // NCORES)
    T_core = -(-T_core // GROUP_T) * GROUP_T
    T_pad = T_core * NCORES
    slots_c = T_core * TILE_S
    n_slots = T_pad * TILE_S

    perm = np.full(n_nodes, -1, np.int64)
    for t, nodes in enumerate(tiles):
        for j, node in enumerate(nodes):
            perm[node] = t * TILE_S + j
    assert (perm >= 0).all()

    e_src_slot = np.zeros((T_pad, TILE_E), np.int16)
    e_dst_loc = np.zeros((T_pad, TILE_E), np.int16)
    onehot = np.zeros((T_pad, TILE_E, 128), np.float32)  # edge -> slot in group
    for t, nodes in enumerate(tiles):
        k = 0
        for j, node in enumerate(nodes):
            for e in range(offs[node], offs[node + 1]):
                e_src_slot[t, k] = perm[ssrc[e]]
                e_dst_loc[t, k] = (t * TILE_S + j) % slots_c
                onehot[t, k, TILE_S * (t % GROUP_T) + j] = 1.0
                k += 1

    return dict(perm=perm, T_core=T_core, slots_c=slots_c, n_slots=n_slots,
                e_src_slot=e_src_slot, e_dst_loc=e_dst_loc, onehot=onehot)


def wrap_idxs(flat_idx):
    n = flat_idx.shape[0]
    cols = -(-n // 16)
    iw = np.zeros((cols, 16), np.int16)
    iw.reshape(-1)[:n] = flat_idx.astype(np.int16)
    return np.tile(np.ascontiguousarray(iw.T), (8, 1))


def prepare_inputs(hidden_states, attention_mask, edge_src, edge_dst,
                   Wq, bq, Wk, bk, Wv, bv, Wo, bo, ln_g, ln_b, st=None):
    x = np.asarray(hidden_states, np.float32).reshape(-1, D)
    n_nodes = x.shape[0]
    if st is None:
        st = build_structures(edge_src, edge_dst, n_nodes)
    perm, slots_c, n_slots = st["perm"], st["slots_c"], st["n_slots"]
    T_core = st["T_core"]

    x_slot = np.zeros((n_slots, D), np.float32)
    x_slot[perm] = x

    WqkvT = np.concatenate([
        np.asarray(Wq, np.float32).T / math.sqrt(HD),
        np.asarray(Wk, np.float32).T,
        np.asarray(Wv, np.float32).T], axis=1)
    bqkv = np.concatenate([
        np.asarray(bq, np.float32) / math.sqrt(HD),
        np.asarray(bk, np.float32),
        np.asarray(bv, np.float32)])[None, :]
    WoT = np.ascontiguousarray(np.asarray(Wo, np.float32).T)
    bo_row = np.asarray(bo, np.float32)[None, :]
    g_rep = np.tile(np.asarray(ln_g, np.float32)[None, :], (128, 1))
    b_rep = np.tile(np.asarray(ln_b, np.float32)[None, :], (128, 1))
    ident = np.eye(128, dtype=np.float32)

    in_maps = []
    for c in range(NCORES):
        sl = slice(c * slots_c, (c + 1) * slots_c)
        tl = slice(c * T_core, (c + 1) * T_core)
        xc = x_slot[sl]
        oh = st["onehot"][tl]                               # [T_core, 128, 128]
        oh_sb = np.ascontiguousarray(oh.transpose(1, 0, 2)).reshape(
            TILE_E, T_core * 128)
        in_maps.append({
            "x_c": xc,
            "xT_c": np.ascontiguousarray(xc.T),
            "wqkvT": WqkvT, "bqkv": bqkv,
            "woT": WoT, "bo_row": bo_row,
            "g_rep": g_rep, "b_rep": b_rep, "ident": ident,
            "src_idx": wrap_idxs(st["e_src_slot"][tl].reshape(-1)),
            "dst_idx": wrap_idxs(st["e_dst_loc"][tl].reshape(-1)),
            "onehot_in": oh_sb.astype(np.float16).view(np.int16),
        })
    return in_maps, dict(st=st)


# ---------------------------------------------------------------------------
# Device program
# ---------------------------------------------------------------------------

def build_program(T_core, slots_c, n_slots, debug=False, dbg_taps=False,
                  collective_proxy=False):
    import concourse.bass as bass
    import concourse.mybir as mybir
    import concourse.tile as tile
    import concourse.bacc as bacc
    from concourse.tile_rust import add_dep_helper

    def dep(after, *befores):
        ai = after.ins if hasattr(after, "ins") else after
        for b in befores:
            if b is None:
                continue
            bi = b.ins if hasattr(b, "ins") else b
            add_dep_helper(ai, bi, reason="manual dma_gather fence")
        return after

    F32, BF16, I16 = mybir.dt.float32, mybir.dt.float16, mybir.dt.int16
    AX = mybir.AxisListType
    ACT = mybir.ActivationFunctionType
    G = T_core // GROUP_T
    E_core = T_core * TILE_E
    ECHUNK = GROUP_T * TILE_E            # MP gather chunk (2048 edges)
    SCHUNK = SCHUNK_T * TILE_E           # score gather chunk (1024 edges)
    HCH_T = 8
    KD = D // 128
    QKV_N = 3 * D
    NB = 384
    rg = [list(range(NCORES))]

    nc = bacc.Bacc("TRN2", target_bir_lowering=False, debug=debug,
                   num_devices=1 if collective_proxy else NCORES)

    def allgather(src_tile, dst_tile):
        if collective_proxy:
            # timing proxy: local HBM copy of the shard (collectives are not
            # modellable in TimelineSim)
            return nc.gpsimd.dma_start(dst_tile[0:slots_c, :], src_tile[:])
        return nc.gpsimd.collective_compute(
            "AllGather", mybir.AluOpType.bypass, replica_groups=rg,
            ins=[src_tile.opt()], outs=[dst_tile.opt()])

    x_c = nc.dram_tensor("x_c", [slots_c, D], F32, kind="ExternalInput")
    xT_c = nc.dram_tensor("xT_c", [D, slots_c], F32, kind="ExternalInput")
    wqkvT = nc.dram_tensor("wqkvT", [D, QKV_N], F32, kind="ExternalInput")
    bqkv_t = nc.dram_tensor("bqkv", [1, QKV_N], F32, kind="ExternalInput")
    woT_t = nc.dram_tensor("woT", [D, D], F32, kind="ExternalInput")
    bo_t = nc.dram_tensor("bo_row", [1, D], F32, kind="ExternalInput")
    g_t = nc.dram_tensor("g_rep", [128, D], F32, kind="ExternalInput")
    b_t = nc.dram_tensor("b_rep", [128, D], F32, kind="ExternalInput")
    id_t = nc.dram_tensor("ident", [128, 128], F32, kind="ExternalInput")
    srcix_t = nc.dram_tensor("src_idx", [128, E_core // 16], I16, kind="ExternalInput")
    dstix_t = nc.dram_tensor("dst_idx", [128, E_core // 16], I16, kind="ExternalInput")
    oh_t = nc.dram_tensor("onehot_in", [TILE_E, T_core * 128], I16,
                          kind="ExternalInput")
    out_c = nc.dram_tensor("out_c", [slots_c, D], F32, kind="ExternalOutput")
    if dbg_taps:
        dbg_pexp = nc.dram_tensor("dbg_pexp", [TILE_E, T_core * H], F32,
                                  kind="ExternalOutput")
        dbg_scale = nc.dram_tensor("dbg_scale", [128, G * H], F32,
                                   kind="ExternalOutput")
        dbg_aex = nc.dram_tensor("dbg_aex", [128, H * HD], F32,
                                 kind="ExternalOutput")
        dbg_h1 = nc.dram_tensor("dbg_h1", [slots_c, D], F32,
                                kind="ExternalOutput")

    with tile.TileContext(nc) as tc, contextlib.ExitStack() as X:
        ep = X.enter_context
        keep = ep(tc.tile_pool(name="keep", bufs=1))       # long-lived small
        sb = ep(tc.tile_pool(name="sb", bufs=2))           # streaming tiles
        one = ep(tc.tile_pool(name="one", bufs=1))         # single-buffered big
        ps1 = ep(tc.tile_pool(name="ps1", bufs=2, space="PSUM"))
        ps2 = ep(tc.tile_pool(name="ps2", bufs=2, space="PSUM"))
        dram = ep(tc.tile_pool(name="dram", bufs=1, space="DRAM"))

        # ---- persistent / index data ----
        src_ix = keep.tile([128, E_core // 16], I16, tag="srcix")
        ld_srcix = nc.sync.dma_start(src_ix[:], srcix_t[:])
        dst_ix = keep.tile([128, E_core // 16], I16, tag="dstix")
        ld_dstix = nc.sync.dma_start(dst_ix[:], dstix_t[:])
        ones_row = keep.tile([1, 128], F32, tag="ones")
        nc.gpsimd.memset(ones_row[:], 1.0)
        eps_t = keep.tile([128, 1], F32, tag="eps")
        nc.gpsimd.memset(eps_t[:], float(EPS))
        idn = keep.tile([128, 128], F32, tag="idn")
        nc.sync.dma_start(idn[:], id_t[:])

        v_bf = keep.tile([128, G, D], BF16, tag="v_bf")    # v rows (slot-major)
        scale_sb = keep.tile([128, G * H], F32, tag="scale")
        scv = scale_sb[:].rearrange("p (g h) -> p g h", g=G, h=H)
        pexp = keep.tile([TILE_E, T_core, H], BF16, tag="pexp")

        # HBM tables
        q_loc = dram.tile([slots_c, D], BF16, tag="q_loc")
        k_shard = dram.tile([slots_c, D], BF16, tag="k_shard")
        v_shard = dram.tile([slots_c, D], BF16, tag="v_shard")
        k_full = dram.tile([n_slots, D], BF16, addr_space="Shared", tag="k_full")
        h_fulls = [dram.tile([n_slots, D], BF16, addr_space="Shared", tag=f"hf{s}",
                             name=f"hf{s}")
                   for s in range(STEPS)]
        h_shards = [dram.tile([slots_c, D], BF16, tag=f"hs{s}", name=f"hs{s}")
                    for s in range(STEPS - 1)]

        # ============================ QKV ============================
        # out[slot-tile g, qkv-col chunk nb] = sum_k xT[k, g]^T @ WqkvT[k, nb]
        # "bigA" tag lifetime: xT (QKV) -> onehot (scores+MP) -> wo (output)
        xT_sb = one.tile([128, KD, slots_c], F32, tag="bigA")
        nc.sync.dma_start(xT_sb[:], xT_c[:].rearrange("(k p) n -> p k n", p=128))
        bq_sb = keep.tile([1, QKV_N], F32, tag="bq")
        nc.sync.dma_start(bq_sb[:], bqkv_t[:])

        qloc_writers = []
        for nb in range(QKV_N // NB):
            cs = slice(nb * NB, (nb + 1) * NB)
            wqnb = one.tile([128, KD, NB], F32, tag="wqnb")
            nc.sync.dma_start(wqnb[:],
                              wqkvT[:, cs].rearrange("(k p) n -> p k n", p=128))
            part = nb * NB // D          # 0=q, 1=k, 2=v
            po = (nb * NB) % D
            for g in range(G):
                acc = ps1.tile([128, NB], F32, tag="qkv_acc")
                for k in range(KD):
                    nc.tensor.matmul(acc[:], xT_sb[:, k, g * 128:(g + 1) * 128],
                                     wqnb[:, k, :], start=(k == 0), stop=False)
                nc.tensor.matmul(acc[:], ones_row[:, :128], bq_sb[:, cs],
                                 start=False, stop=True)
                ev = sb.tile([128, NB], BF16, tag="ev")
                nc.vector.tensor_copy(ev[:], acc[:])
                tgt = (q_loc, k_shard, v_shard)[part]
                winst = nc.sync.dma_start(tgt[g * 128:(g + 1) * 128, po:po + NB],
                                          ev[:])
                if part == 0:
                    qloc_writers.append(winst)
                if part == 2:
                    nc.vector.tensor_copy(v_bf[:, g, po:po + NB], acc[:])

        ag_k = allgather(k_shard, k_full)
        ag_h = allgather(v_shard, h_fulls[0])

        # ========================== scores ===========================
        oh_sb = one.tile([TILE_E, T_core * 128], I16, tag="bigA")
        nc.sync.dma_start(oh_sb[:], oh_t[:])
        ohv = oh_sb[:].bitcast(BF16).rearrange("p (t s) -> p t s", t=T_core, s=128)

        # manually double-buffered gather tiles (Tile cannot track dma_gather)
        gbufA = [keep.tile([128, SCHUNK_T, D], BF16, tag="gbufA", name="gbufA"),
                 keep.tile([128, SCHUNK_T, D], BF16, tag="gbufA2", name="gbufA2")]
        gbufB = [keep.tile([128, SCHUNK_T, D], BF16, tag="gbufB", name="gbufB"),
                 keep.tile([128, SCHUNK_T, D], BF16, tag="gbufB2", name="gbufB2")]
        lastA = [None, None]
        lastB = [None, None]

        for sch in range(E_core // SCHUNK):
            kg, qg = gbufA[sch % 2], gbufB[sch % 2]
            io = slice(sch * SCHUNK // 16, (sch + 1) * SCHUNK // 16)
            g1 = dep(nc.gpsimd.dma_gather(kg[:], k_full[:], src_ix[:, io],
                                          SCHUNK, SCHUNK, D),
                     ld_srcix, ag_k, lastA[sch % 2])
            g2 = dep(nc.gpsimd.dma_gather(qg[:], q_loc[:], dst_ix[:, io],
                                          SCHUNK, SCHUNK, D),
                     ld_dstix, lastB[sch % 2], *qloc_writers)
            tt = dep(nc.vector.tensor_mul(kg[:], kg[:], qg[:]), g1, g2)
            lastB[sch % 2] = tt
            sc = sb.tile([128, SCHUNK_T * H], F32, tag="sc")
            red = nc.vector.tensor_reduce(
                sc[:], kg[:].rearrange("p t (h d) -> p (t h) d", h=H, d=HD),
                axis=AX.X, op=mybir.AluOpType.add)
            lastA[sch % 2] = red
            ts = slice(sch * SCHUNK_T, (sch + 1) * SCHUNK_T)
            nc.scalar.activation(
                pexp[:, ts, :].rearrange("p t h -> p (t h)"), sc[:], ACT.Exp)

        # denominators -> scale = 0.9/denom
        for g in range(G):
            dacc = ps1.tile([128, H], F32, tag="qkv_acc")
            for t16 in range(GROUP_T):
                t = g * GROUP_T + t16
                nc.tensor.matmul(dacc[:], ohv[:, t, :], pexp[:, t, :],
                                 start=(t16 == 0), stop=(t16 == GROUP_T - 1))
            nc.vector.tensor_copy(scv[:, g, :], dacc[:])
        nc.vector.tensor_scalar_max(scale_sb[:], scale_sb[:], 1e-30)
        nc.vector.reciprocal(scale_sb[:], scale_sb[:])
        nc.scalar.mul(scale_sb[:], scale_sb[:], 1.0 - ALPHA)

        if dbg_taps:
            dpx = sb.tile([TILE_E, T_core * H], F32, tag="dpx", name="dpx", bufs=1)
            nc.vector.tensor_copy(dpx[:], pexp[:].rearrange("p t h -> p (t h)"))
            nc.sync.dma_start(dbg_pexp[:], dpx[:])
            dsc = sb.tile([128, G * H], F32, tag="dsc", name="dsc", bufs=1)
            nc.vector.tensor_copy(dsc[:], scale_sb[:])
            nc.sync.dma_start(dbg_scale[:], dsc[:])

        # ======================= message passing =====================
        hnew = None
        nchunk = 0
        for step in range(STEPS):
            last = step == STEPS - 1
            ag_prev = ag_h
            hnew = one.tile([128, G, D], F32, tag="hnew", name="hnew")
            for g in range(G):
                agg = ps2.tile([128, D], F32, tag="agg")
                for half in range(GROUP_T // HCH_T):
                    gt = gbufA[nchunk % 2]
                    c0 = g * GROUP_T + half * HCH_T
                    io = slice(c0 * TILE_E // 16, (c0 + HCH_T) * TILE_E // 16)
                    gi = dep(nc.gpsimd.dma_gather(gt[:], h_fulls[step][:],
                                                  src_ix[:, io],
                                                  HCH_T * TILE_E, HCH_T * TILE_E,
                                                  D),
                             ld_srcix, ag_prev, lastA[nchunk % 2])
                    msg = gbufB[nchunk % 2]
                    last_tt = None
                    for t8 in range(HCH_T):
                        t = c0 + t8
                        aex = sb.tile([128, H * HD], BF16, tag="aex")
                        nc.scalar.activation(
                            aex[:].rearrange("p (h d) -> p h d", h=H, d=HD),
                            pexp[:, t, :].rearrange("p h -> p h ()")
                                .broadcast_to([128, H, HD]),
                            ACT.Copy)
                        if dbg_taps and step == 0 and g == 0 and half == 0 and t8 == 0:
                            dax = sb.tile([128, H * HD], F32, tag="dax",
                                          name="dax", bufs=1)
                            nc.vector.tensor_copy(dax[:], aex[:])
                            nc.sync.dma_start(dbg_aex[:], dax[:])
                        last_tt = dep(
                            nc.vector.tensor_mul(msg[:, t8, :], gt[:, t8, :],
                                                 aex[:]), gi)
                        t16 = half * HCH_T + t8
                        for c0_, cw_ in ((0, 512), (512, 256)):
                            cs = slice(c0_, c0_ + cw_)
                            nc.tensor.matmul(agg[:, cs], ohv[:, t, :],
                                             msg[:, t8, cs],
                                             start=(t16 == 0),
                                             stop=(t16 == GROUP_T - 1))
                    lastA[nchunk % 2] = last_tt
                    nchunk += 1
                nc.vector.tensor_copy(hnew[:, g, :], agg[:])
                for h in range(H):
                    nc.vector.tensor_scalar_mul(
                        hnew[:, g, h * HD:(h + 1) * HD],
                        hnew[:, g, h * HD:(h + 1) * HD], scv[:, g, h:h + 1])
                v10g = sb.tile([128, D], F32, tag="v10g")
                nc.scalar.activation(v10g[:], v_bf[:, g, :], ACT.Copy,
                                     scale=ALPHA)
                nc.vector.tensor_add(hnew[:, g, :], hnew[:, g, :], v10g[:])
                if not last:
                    hb = sb.tile([128, D], BF16, tag="ev")
                    nc.vector.tensor_copy(hb[:], hnew[:, g, :])
                    nc.sync.dma_start(h_shards[step][g * 128:(g + 1) * 128, :],
                                      hb[:])
            if dbg_taps and step == 0:
                nc.gpsimd.dma_start(dbg_h1[:], h_shards[0][:])
            if not last:
                ag_h = allgather(h_shards[step], h_fulls[step + 1])

        # ========================== output ===========================
        wo_sb = one.tile([128, KD, D], F32, tag="bigA")
        nc.sync.dma_start(wo_sb[:], woT_t[:].rearrange("(k p) n -> p k n", p=128))
        bo_sb = keep.tile([1, D], F32, tag="bo")
        nc.sync.dma_start(bo_sb[:], bo_t[:])
        gam = sb.tile([128, D], F32, tag="gam", bufs=1)
        nc.sync.dma_start(gam[:], g_t[:])
        bet = sb.tile([128, D], F32, tag="bet", bufs=1)
        nc.sync.dma_start(bet[:], b_t[:])

        for g in range(G):
            tp = ps2.tile([128, D], F32, tag="agg")
            for k in range(KD):
                nc.tensor.transpose(tp[:, k * 128:(k + 1) * 128],
                                    hnew[:, g, k * 128:(k + 1) * 128], idn[:])
            h5T = sb.tile([128, KD, 128], F32, tag="h5T", bufs=1)
            nc.vector.tensor_copy(h5T[:], tp[:].rearrange("p (k q) -> p k q", k=KD))
            yac = ps2.tile([128, D], F32, tag="agg")
            for c0_, cw_ in ((0, 512), (512, 256)):
                cs = slice(c0_, c0_ + cw_)
                for k in range(KD):
                    nc.tensor.matmul(yac[:, cs], h5T[:, k, :], wo_sb[:, k, cs],
                                     start=(k == 0), stop=False)
                nc.tensor.matmul(yac[:, cs], ones_row[:, :128], bo_sb[:, cs],
                                 start=False, stop=True)
            y = sb.tile([128, D], F32, tag="y")
            nc.vector.tensor_copy(y[:], yac[:])
            x_g = sb.tile([128, D], F32, tag="x_g")
            nc.sync.dma_start(x_g[:], x_c[g * 128:(g + 1) * 128, :])
            nc.vector.tensor_add(y[:], y[:], x_g[:])
            mu = sb.tile([128, 1], F32, tag="mu")
            nc.vector.tensor_reduce(mu[:], y[:], axis=AX.X, op=mybir.AluOpType.add)
            nc.scalar.mul(mu[:], mu[:], 1.0 / D)
            yc = sb.tile([128, D], F32, tag="yc")
            nc.vector.tensor_scalar_sub(yc[:], y[:], mu[:])
            y2 = sb.tile([128, D], F32, tag="sc")
            nc.vector.tensor_mul(y2[:], yc[:], yc[:])
            var = sb.tile([128, 1], F32, tag="var")
            nc.vector.tensor_reduce(var[:], y2[:], axis=AX.X, op=mybir.AluOpType.add)
            rstd = sb.tile([128, 1], F32, tag="rstd")
            nc.scalar.activation(rstd[:], var[:], ACT.Sqrt,
                                 scale=1.0 / D, bias=eps_t[:])
            nc.vector.reciprocal(rstd[:], rstd[:])
            nc.vector.tensor_scalar_mul(yc[:], yc[:], rstd[:])
            nc.vector.tensor_mul(yc[:], yc[:], gam[:])
            nc.vector.tensor_add(yc[:], yc[:], bet[:])
            nc.sync.dma_start(out_c[g * 128:(g + 1) * 128, :], yc[:])

    nc.compile()
    return nc


# ---------------------------------------------------------------------------
# Entry point
# ---------------------------------------------------------------------------

_CACHE = {}


def estimate_device_ns(st):
    from concourse.timeline_sim import TimelineSim
    nc = build_program(st["T_core"], st["slots_c"], st["n_slots"],
                       collective_proxy=True)
    tl = TimelineSim(nc)
    return int(tl.simulate())


def kernel(**inputs):
    from concourse.bass_utils import run_bass_kernel_spmd

    in_maps, meta = prepare_inputs(**inputs)
    st = meta["st"]
    key = (st["T_core"], st["slots_c"], st["n_slots"])
    if key not in _CACHE:
        _CACHE[key] = build_program(*key)
    nc = _CACHE[key]
    res = run_bass_kernel_spmd(nc, in_maps, list(range(NCORES)))
    outs = np.concatenate([res.results[c]["out_c"] for c in range(NCORES)], axis=0)
    full = outs[st["perm"]]
    return np.ascontiguousarray(full.reshape(B, S, D), dtype=np.float32)

